# revision 1
# baseline (speedup 1.0000x reference)
"""Trainium2 Bass kernel for nn_DA_conv (degradation-aware dynamic-filter conv).

kernel(**inputs) takes FULL inputs (as from setup_inputs()), shards batch
B=16 across 8 NeuronCores (2 batches/core), runs one SPMD Bass program on
cores 0-7, gathers the full [16,64,128,128] fp32 output.

Per-core layout: channel-major. Partitions = (batch_pair 2 x C 64) = 128,
free dim = H*W pixels, processed in 8 chunks of 16 image rows.

  kc   = (leaky(x1@Wkc1)@Wkc2), att' = 1+sigmoid(...)   -> host (tiny)
  hidden = leaky(Wks1^T x2 + bks1)                      -> PE + ACT(Lrelu)
  ksp  = Wks2^T hidden + bks2  (bf16)                   -> PE + ACT
  ddf  = sum_ij kc[c,ij]*ksp[ij,hw]*x0[c,hw+shift_ij]   -> DVE bf16 STT/TT
         (ksp replicated across the 64 channel partitions via DMA)
  mx   = max_c x0  -> DVE 6-level partition fold (bf16)
  av   = mean_c x0 -> PE ones-matmul
  sa   = sigmoid(36-tap conv over [mx;av] + bsa)        -> PE + ACT
  out  = Wconv^T ddf + bconv + x0*(att'[c] + sa[hw])    -> PE + DVE
"""

import sys

sys.path.insert(0, "/opt/trn_rl_repo")

import numpy as np
import ml_dtypes

import concourse.bass as bass
import concourse.tile as tile
from concourse import bacc, mybir
from concourse.bass_utils import run_bass_kernel_spmd

F32 = mybir.dt.float32
BF16 = mybir.dt.bfloat16
AF = mybir.ActivationFunctionType
OP = mybir.AluOpType

B, C, H, W = 16, 64, 128, 128
KK = 9
HW = H * W
NCORES = 8
BPC = B // NCORES          # batches per core
RC = 16                    # image rows per chunk
NCH = H // RC              # 8 chunks
F = RC * W                 # 2048 pixels per chunk
SUB = 512
NSUB = F // SUB
PW = 132                   # padded row width (image col w -> pad col w+2)
PR = 130                   # padded rows      (image row h -> pad row h+1)
PADN = PR * PW


def _leaky(v):
    return np.where(v >= 0, v, 0.1 * v)


def _build_program():
    nc = bacc.Bacc("TRN2", target_bir_lowering=False, debug=False,
                   num_devices=NCORES)

    def din(name, shape, dt=F32):
        return nc.dram_tensor(name, shape, dt, kind="ExternalInput").ap()

    x0_d = din("x0", [128, HW])
    x2_d = din("x2", [128, HW])
    kcsel_d = din("kcsel", [41, KK * 64], BF16)
    attp_d = din("attp", [128, 1])
    bconv_d = din("bconv2", [128, 1])
    wks1_d = din("wks1", [64, 64])
    wks2_d = din("wks2", [64, KK])
    bks1_d = din("bks1", [128, 1])
    bks2_d = din("bks2", [41, 1])
    wconv_d = din("wconv", [64, 64], BF16)
    wmean_d = din("wmean", [64, 1], BF16)
    wsel2_d = din("wsel2", [2, 128], BF16)
    wsa_d = din("wsa", [12, 6], BF16)
    bsa_d = din("bsa", [2, 1])
    out_d = nc.dram_tensor("out", [128, HW], F32, kind="ExternalOutput").ap()

    with tile.TileContext(nc) as tc:
        with (
            tc.tile_pool(name="persist", bufs=1) as pp,
            tc.tile_pool(name="st2", bufs=2) as st2,
            tc.tile_pool(name="st3", bufs=3) as st3,
            tc.tile_pool(name="wk1", bufs=1) as wk1,
            tc.tile_pool(name="outp", bufs=2) as outp,
            tc.tile_pool(name="ps_misc", bufs=4, space=bass.MemorySpace.PSUM) as psm,
            tc.tile_pool(name="ps_f", bufs=1, space=bass.MemorySpace.PSUM) as psf,
        ):
            # ---- persistent SBUF ----
            pad1 = pp.tile([128, PADN], BF16)
            pad2 = pp.tile([128, PADN], BF16)
            # mx/av maps at 32-aligned partitions: 0=mx_b0, 32=av_b0,
            # 64=mx_b1, 96=av_b1 (engine ops need 32-aligned bases)
            # + PW cols: flat 17-row tap windows over-read past row 129
            mxav = pp.tile([97, PADN + PW], BF16)
            vscr = pp.tile([128, RC * PW], BF16)   # fold staging (memset once)
            wks1_sb = pp.tile([128, 64], F32)
            wks2_sb = pp.tile([128, KK], F32)
            wconv_sb = pp.tile([128, 64], BF16)
            wmean_sb = pp.tile([128, 1], BF16)
            wsel2_sb = pp.tile([2, 128], BF16)
            wsa_sb = pp.tile([12, 6], BF16)
            kcsel_sb = pp.tile([41, KK * 64], BF16)
            attp_sb = pp.tile([128, 1], F32)
            bconv_sb = pp.tile([128, 1], F32)
            bks1_sb = pp.tile([128, 1], F32)
            bks2_sb = pp.tile([41, 1], F32)
            bsa_sb = pp.tile([2, 1], F32)

            dma = nc.sync.dma_start
            dmas = nc.scalar.dma_start
            dma(wks1_sb[0:64, :], wks1_d[:])
            dma(wks1_sb[64:128, :], wks1_d[:])
            dma(wks2_sb[0:64, :], wks2_d[:])
            dma(wks2_sb[64:128, :], wks2_d[:])
            dma(wconv_sb[0:64, :], wconv_d[:])
            dma(wconv_sb[64:128, :], wconv_d[:])
            dma(wmean_sb[0:64, :], wmean_d[:])
            dma(wmean_sb[64:128, :], wmean_d[:])
            dma(wsel2_sb[:], wsel2_d[:])
            dma(wsa_sb[:], wsa_d[:])
            dma(kcsel_sb[:], kcsel_d[:])
            dma(attp_sb[:], attp_d[:])
            dma(bconv_sb[:], bconv_d[:])
            dma(bks1_sb[:], bks1_d[:])
            dma(bks2_sb[:], bks2_d[:])
            dma(bsa_sb[:], bsa_d[:])

            nc.vector.memset(pad1[:], 0.0)
            nc.vector.memset(mxav[:], 0.0)
            nc.vector.memset(vscr[:], 0.0)

            p1v = pad1.rearrange("p (r w) -> p r w", w=PW)
            p2v = pad2.rearrange("p (r w) -> p r w", w=PW)

            # ======= phase A: pad build + channel max/mean, all chunks =======
            for ch in range(NCH):
                r0 = RC * ch
                base = (r0 + 1) * PW
                n = RC * PW

                x0st = st2.tile([128, F], F32, tag="x0st")
                dma(x0st[:], x0_d[:, ch * F:(ch + 1) * F])
                nc.vector.tensor_copy(
                    p1v[:, r0 + 1:r0 + 17, 2:130],
                    x0st.rearrange("p (r w) -> p r w", w=W),
                )
                lo = 0 if ch == 0 else base
                hi = PADN if ch == NCH - 1 else base + n
                dma(pad2[:, lo + 1:hi], pad1[:, lo:hi - 1])

                # channel max: partition fold. TensorTensor requires both SBUF
                # inputs at the SAME start partition, so hi-halves are DMA
                # staged into vscr (memset once; middle reads see stale zeros
                # that never reach valid output rows).
                s1 = wk1.tile([128, n], BF16, tag="mxs1")
                s2 = wk1.tile([128, n], BF16, tag="mxs2")
                dmas(vscr[0:32, :], pad1[32:64, base:base + n])
                dmas(vscr[64:96, :], pad1[96:128, base:base + n])
                nc.vector.tensor_max(s1[0:96, :], pad1[0:96, base:base + n],
                                     vscr[0:96, :])
                cur, nxt = s1, s2
                for k in (16, 8, 4, 2, 1):
                    dmas(vscr[0:k, :], cur[k:2 * k, :])
                    dmas(vscr[64:64 + k, :], cur[64 + k:64 + 2 * k, :])
                    nc.vector.tensor_max(nxt[0:64 + k, :], cur[0:64 + k, :],
                                         vscr[0:64 + k, :])
                    cur, nxt = nxt, cur
                dmas(mxav[0:1, base:base + n], cur[0:1, :])
                dmas(mxav[64:65, base:base + n], cur[64:65, :])

                # channel mean via PE ones-matmul, ACT evac (bf16 cast)
                for s0 in range(0, n, SUB):
                    ln = min(SUB, n - s0)
                    avps = psm.tile([33, SUB], F32, tag="ps1")
                    nc.tensor.matmul(avps[0:1, 0:ln], wmean_sb[0:64, :],
                                     pad1[0:64, base + s0:base + s0 + ln],
                                     start=True, stop=True, tile_position=(0, 0))
                    nc.tensor.matmul(avps[32:33, 0:ln], wmean_sb[64:128, :],
                                     pad1[64:128, base + s0:base + s0 + ln],
                                     start=True, stop=True, tile_position=(64, 32))
                    nc.scalar.activation(mxav[32:33, base + s0:base + s0 + ln],
                                         avps[0:1, 0:ln], AF.Copy)
                    nc.scalar.activation(mxav[96:97, base + s0:base + s0 + ln],
                                         avps[32:33, 0:ln], AF.Copy)

            # ======= phase B: ksp, ddf combine, sa, final =======
            for ch in range(NCH):
                r0 = RC * ch
                cslice = slice(ch * F, (ch + 1) * F)

                # hidden = leaky(Wks1^T x2 + bks1); ksp = Wks2^T hidden + bks2
                ksp_sb = st2.tile([41, F], BF16, tag="ksp")
                for h2 in range(2):
                    x2t = st2.tile([128, F // 2], F32, tag="x2t")
                    dma(x2t[:], x2_d[:, ch * F + h2 * (F // 2):
                                 ch * F + (h2 + 1) * (F // 2)])
                    for s2i in range(2):
                        ssl = slice((2 * h2 + s2i) * SUB, (2 * h2 + s2i + 1) * SUB)
                        xsl = slice(s2i * SUB, (s2i + 1) * SUB)
                        hps = psm.tile([128, SUB], F32, tag="ps1")
                        nc.tensor.matmul(hps[0:64, :], wks1_sb[0:64, :],
                                         x2t[0:64, xsl], start=True, stop=True,
                                         tile_position=(0, 0))
                        nc.tensor.matmul(hps[64:128, :], wks1_sb[64:128, :],
                                         x2t[64:128, xsl], start=True, stop=True,
                                         tile_position=(64, 64))
                        hsb = st3.tile([128, SUB], F32, tag="hsb")
                        nc.scalar.activation(hsb[:], hps[:], AF.Lrelu,
                                             bias=bks1_sb[:, 0:1], alpha=0.1)
                        kps = psm.tile([41, SUB], F32, tag="ps1")
                        nc.tensor.matmul(kps[0:9, :], wks2_sb[0:64, :],
                                         hsb[0:64, :], start=True, stop=True,
                                         tile_position=(0, 0))
                        nc.tensor.matmul(kps[32:41, :], wks2_sb[64:128, :],
                                         hsb[64:128, :], start=True, stop=True,
                                         tile_position=(64, 32))
                        nc.scalar.activation(ksp_sb[0:9, ssl], kps[0:9, :],
                                             AF.Identity, bias=bks2_sb[0:9, 0:1])
                        nc.scalar.activation(ksp_sb[32:41, ssl], kps[32:41, :],
                                             AF.Identity,
                                             bias=bks2_sb[32:41, 0:1])

                # ddf: per ij, broadcast kc*ksp across channel partitions via
                # one-hot selector matmuls, z = shift(x0) * field, and fold the
                # ij-sum into convF's PSUM accumulation.
                fps = psf.tile([128, F], F32, tag="fps")
                for ij in range(KK):
                    i, j = divmod(ij, 3)
                    if j == 1:
                        srcv, joff = p1v, 2
                    else:
                        srcv, joff = p2v, j + 2
                    x0s = srcv[:, r0 + i:r0 + i + RC, joff:joff + W]
                    ksl = slice(ij * 64, (ij + 1) * 64)
                    kbc2 = st2.tile([128, F], BF16, tag="kbc2")
                    for s in range(NSUB):
                        ssl = slice(s * SUB, (s + 1) * SUB)
                        kbcps = psm.tile([128, SUB], F32, tag="ps1")
                        nc.tensor.matmul(kbcps[0:64, :], kcsel_sb[0:9, ksl],
                                         ksp_sb[0:9, ssl], start=True,
                                         stop=True, tile_position=(0, 0))
                        nc.tensor.matmul(kbcps[64:128, :], kcsel_sb[32:41, ksl],
                                         ksp_sb[32:41, ssl], start=True,
                                         stop=True, tile_position=(32, 64))
                        if s < 2:
                            nc.scalar.activation(kbc2[:, ssl], kbcps[:],
                                                 AF.Copy)
                        else:
                            nc.vector.tensor_copy(kbc2[:, ssl], kbcps[:])
                    z = st2.tile([128, F], BF16, tag="z")
                    nc.vector.tensor_mul(z.rearrange("p (r w) -> p r w", w=W),
                                         x0s,
                                         kbc2.rearrange("p (r w) -> p r w", w=W))
                    for s in range(NSUB):
                        ssl = slice(s * SUB, (s + 1) * SUB)
                        nc.tensor.matmul(fps[0:64, ssl], wconv_sb[0:64, :],
                                         z[0:64, ssl], start=(ij == 0),
                                         stop=(ij == KK - 1),
                                         tile_position=(0, 0),
                                         skip_group_check=True)
                        nc.tensor.matmul(fps[64:128, ssl], wconv_sb[64:128, :],
                                         z[64:128, ssl], start=(ij == 0),
                                         stop=(ij == KK - 1),
                                         tile_position=(64, 64),
                                         skip_group_check=True)

                # spatial attention: 12 wide tap rows (g,i); 3 accumulating
                # matmuls (one per j shift) then sigmoid
                nt = 17 * PW
                tap = wk1.tile([12, nt], BF16, tag="tap")
                for g, mrow in enumerate((0, 32, 64, 96)):
                    for i in range(3):
                        o = (r0 + i) * PW
                        dmas(tap[3 * g + i:3 * g + i + 1, :],
                             mxav[mrow:mrow + 1, o:o + nt])
                tapv = tap.rearrange("p (r w) -> p r w", w=PW)
                sasb = st2.tile([2, F], BF16, tag="sasb", bufs=1)
                for s in range(NSUB):
                    saps = psm.tile([128, SUB], F32, tag="ps1")
                    for j in range(3):
                        nc.tensor.matmul(saps[0:2, :],
                                         wsa_sb[0:12, 2 * j:2 * j + 2],
                                         tapv[:, 4 * s:4 * s + 4,
                                              j + 1:j + 1 + W],
                                         start=(j == 0), stop=(j == 2),
                                         tile_position=(0, 0))
                    nc.scalar.activation(sasb[0:2, s * SUB:(s + 1) * SUB],
                                         saps[0:2, :], AF.Sigmoid,
                                         bias=bsa_sb[:, 0:1])

                # final: out = Wconv^T ddf + bconv + x0*(att' + sa)
                x0re = st2.tile([128, F], F32, tag="x0st")
                dma(x0re[:], x0_d[:, cslice])
                outf = outp.tile([128, F], F32, tag="outf")
                for s in range(NSUB):
                    ssl = slice(s * SUB, (s + 1) * SUB)
                    bps = psm.tile([128, SUB], F32, tag="ps1")
                    nc.tensor.matmul(bps[0:64, :], wsel2_sb[:, 0:64],
                                     sasb[:, ssl], start=True, stop=True,
                                     tile_position=(0, 0))
                    nc.tensor.matmul(bps[64:128, :], wsel2_sb[:, 64:128],
                                     sasb[:, ssl], start=True, stop=True,
                                     tile_position=(0, 64))
                    t1 = outp.tile([128, SUB], F32, tag="t1")
                    nc.vector.scalar_tensor_tensor(
                        t1[:], bps[:], attp_sb[:, 0:1], x0re[:, ssl],
                        OP.add, OP.mult)
                    nc.vector.scalar_tensor_tensor(
                        outf[:, ssl], fps[:, ssl], bconv_sb[:, 0:1], t1[:],
                        OP.add, OP.add)
                dma(out_d[:, cslice], outf[:])

    nc.compile()
    return nc


_CACHED = {}


def _get_program():
    if "nc" not in _CACHED:
        _CACHED["nc"] = _build_program()
    return _CACHED["nc"]


def make_in_maps(x0, x1, x2, Wkc1, Wkc2, Wks1, bks1, Wks2, bks2,
                 Wconv, bconv, Wca1, Wca2, Wsa, bsa):
    x0 = np.asarray(x0, np.float32)
    x1 = np.asarray(x1, np.float32)
    x2 = np.asarray(x2, np.float32)

    kc = (_leaky(x1 @ np.asarray(Wkc1, np.float32))
          @ np.asarray(Wkc2, np.float32)).reshape(B, C, KK)
    att = 1.0 / (1.0 + np.exp(-(_leaky(x1 @ np.asarray(Wca1, np.float32))
                                @ np.asarray(Wca2, np.float32))))
    attp = (1.0 + att).astype(np.float32)

    bf = ml_dtypes.bfloat16
    wconv_b = np.asarray(Wconv, np.float32).astype(bf)
    wmean_b = np.full((64, 1), 1.0 / 64.0, dtype=bf)
    # sa_bc selector: out[c in 0:64] <- sa row 0 (b0); out[c in 64:128] <- row 1
    wsel2_b = np.zeros((2, 128), dtype=bf)
    wsel2_b[0, 0:64] = 1.0
    wsel2_b[1, 64:128] = 1.0
    # sa taps: one [12, 2] block per j shift; rows (g, i) with g=(b, src)
    wsa_np = np.asarray(Wsa, np.float32)[0]          # [2, 3, 3]
    wsa_b = np.zeros((12, 6), dtype=np.float32)
    for jj in range(3):
        for g in range(4):
            b, srcm = divmod(g, 2)
            for ii in range(3):
                wsa_b[3 * g + ii, 2 * jj + b] = wsa_np[srcm, ii, jj]
    wsa_b = wsa_b.astype(bf)
    bks2_np = np.zeros((41, 1), np.float32)
    bks2_np[0:9, 0] = np.asarray(bks2, np.float32)
    bks2_np[32:41, 0] = np.asarray(bks2, np.float32)
    bks1_np = np.tile(np.asarray(bks1, np.float32), 2).reshape(128, 1)
    bsa_np = np.full((2, 1), float(np.asarray(bsa, np.float32)[0]), np.float32)
    bconv_np = np.ascontiguousarray(
        np.tile(np.asarray(bconv, np.float32), 2).reshape(128, 1))

    shared = {
        "wks1": np.ascontiguousarray(np.asarray(Wks1, np.float32)),
        "wks2": np.ascontiguousarray(np.asarray(Wks2, np.float32)),
        "bks1": bks1_np, "bks2": bks2_np,
        "wconv": wconv_b, "wmean": wmean_b, "wsel2": wsel2_b,
        "wsa": wsa_b, "bsa": bsa_np, "bconv2": bconv_np,
    }

    in_maps = []
    for cid in range(NCORES):
        bsl = slice(BPC * cid, BPC * (cid + 1))
        m = dict(shared)
        m["x0"] = np.ascontiguousarray(x0[bsl].reshape(128, HW))
        m["x2"] = np.ascontiguousarray(x2[bsl].reshape(128, HW))
        # per-(b, ij) one-hot selector columns carrying kc: rows 0-8 (b0) and
        # 32-40 (b1); kcsel[k, ij*64 + c] = kc[b, c, ij] if k == ij else 0
        kcs = np.zeros((41, KK * 64), np.float32)
        for bb in range(BPC):
            r = 32 * bb
            for ij in range(KK):
                kcs[r + ij, ij * 64:(ij + 1) * 64] = kc[BPC * cid + bb, :, ij]
        m["kcsel"] = kcs.astype(bf)
        m["attp"] = np.ascontiguousarray(attp[bsl].reshape(128, 1))
        in_maps.append(m)
    return in_maps


def kernel(**inputs):
    in_maps = make_in_maps(**inputs)
    nc = _get_program()
    res = run_bass_kernel_spmd(nc, in_maps, list(range(NCORES)))
    out = np.empty((B, C, H, W), np.float32)
    for cid in range(NCORES):
        out[BPC * cid:BPC * (cid + 1)] = \
            res.results[cid]["out"].reshape(BPC, C, H, W)
    return out


if __name__ == "__main__":
    _get_program()
    print("program built and compiled OK")

